# revision 2
# baseline (speedup 1.0000x reference)
"""Trainium2 Bass kernel for nn_MultiHeadAttention_5059471475068.

Reference computation (B=2, N=2048, DIM=1024, H=16 heads, d=64):
    q = x @ Wq.T + bq ; k = x @ Wk.T + bk ; v = x @ Wv.T + bv   (per-head split)
    scores[h,b,n,m] = (k[h,b,n,:] . q[h,b,m,:]) / sqrt(DIM)
    attn = softmax(scores, axis=m)
    out[h,b,n,:] = attn @ v ; out = concat_heads @ Wo.T + bo

Sharding: 8 cores = 2 batches x 4 head-groups (4 heads per core).
Each core computes its heads' q,k,v projections, attention, and a partial
output projection (its heads' columns of the concat times the matching rows
of Wo.T). Host sums the 4 partials per batch and adds bo (the unshard step
for the tensor-parallel dimension).

On-chip layout: scores are computed transposed (S^T[m, n], partition = m) so
that softmax-normalized E = exp(S^T) feeds the attn@v matmul directly as the
moving operand with contraction over m, with no transposes.  The softmax
denominator (column sum of E) is folded into the attn@v matmul by appending
a ones-column to v (stationary operand [v | 1], M=65): PSUM row 64 of the
attn@v output accumulates sum_m E[m, n] exactly in fp32.
"""

import sys

if "/opt/trn_rl_repo" not in sys.path:
    sys.path.insert(0, "/opt/trn_rl_repo")

import numpy as np
import ml_dtypes

import concourse.bacc as bacc
import concourse.tile as tile
import concourse.mybir as mybir
from concourse.bass_utils import run_bass_kernel_spmd

BF16 = mybir.dt.bfloat16
F32 = mybir.dt.float32
NPBF16 = ml_dtypes.bfloat16

DIM = 1024
HEADS = 16
HEAD_DIM = 64
B, N = 2, 2048
SCALE = 1.0 / float(np.sqrt(np.float32(DIM)))

N_CORES = 8
GROUPS = 4          # head-groups (one per core within a batch)
HPG = HEADS // GROUPS  # heads per group = 4
DG = HPG * HEAD_DIM    # feature columns per group = 256

KC = DIM // 128     # contraction chunks over features = 8
MT = N // 128       # token tiles = 16
NB = N // 512       # 512-wide column blocks = 4
FT = DIM // 128     # output-feature tiles = 8


def build_kernel():
    """Build the per-core Bass program (identical on all cores; data differs)."""
    nc = bacc.Bacc("TRN2", target_bir_lowering=False, debug=False,
                   num_devices=N_CORES)

    xT = nc.dram_tensor("xT", [DIM, N], BF16, kind="ExternalInput")
    wqT = nc.dram_tensor("wqT", [DIM, DG], BF16, kind="ExternalInput")
    wkT = nc.dram_tensor("wkT", [DIM, DG], BF16, kind="ExternalInput")
    wvT = nc.dram_tensor("wvT", [DIM, DG], BF16, kind="ExternalInput")
    woT = nc.dram_tensor("woT", [DG, DIM], BF16, kind="ExternalInput")
    bq = nc.dram_tensor("bq", [1, DG], BF16, kind="ExternalInput")
    bk = nc.dram_tensor("bk", [1, DG], BF16, kind="ExternalInput")
    bv = nc.dram_tensor("bv", [1, DG], BF16, kind="ExternalInput")
    outT = nc.dram_tensor("outT", [DIM, N], F32, kind="ExternalOutput")

    with tile.TileContext(nc) as tc:
        _body(nc, tc, xT, wqT, wkT, wvT, woT, bq, bk, bv, outT)

    nc.compile()
    return nc


def _body(nc, tc, xT, wqT, wkT, wvT, woT, bq, bk, bv, outT):
    from contextlib import ExitStack

    with ExitStack() as ctx:
        persist = ctx.enter_context(tc.tile_pool(name="persist", bufs=1))

        # --- load inputs ---------------------------------------------------
        xt_sb = []
        for kc in range(KC):
            t = persist.tile([128, N], BF16, tag=f"xt{kc}", name=f"xt{kc}")
            nc.sync.dma_start(out=t[:], in_=xT.ap()[kc * 128:(kc + 1) * 128, :])
            xt_sb.append(t)
        w_sb = {}
        for name, w in (("q", wqT), ("k", wkT), ("v", wvT)):
            for kc in range(KC):
                t = persist.tile([128, DG], BF16, tag=f"w{name}{kc}", name=f"w{name}{kc}")
                nc.sync.dma_start(out=t[:], in_=w.ap()[kc * 128:(kc + 1) * 128, :])
                w_sb[name, kc] = t
        wo_sb = []
        for pc in range(2):
            t = persist.tile([128, DIM], BF16, tag=f"wo{pc}", name=f"wo{pc}")
            nc.sync.dma_start(out=t[:], in_=woT.ap()[pc * 128:(pc + 1) * 128, :])
            wo_sb.append(t)
        b_sb = {}
        for name, bias in (("q", bq), ("k", bk), ("v", bv)):
            t = persist.tile([1, DG], BF16, tag=f"b{name}", name=f"b{name}")
            nc.sync.dma_start(out=t[:], in_=bias.ap()[:, :])
            b_sb[name] = t
        ones = persist.tile([1, N], BF16, tag="ones", name="ones")
        nc.vector.memset(ones[:], 1.0)

        # persistent activations
        qT_sb = [persist.tile([128, N], BF16, tag=f"qT{p}", name=f"qT{p}") for p in range(2)]
        kT_sb = [persist.tile([128, N], BF16, tag=f"kT{p}", name=f"kT{p}") for p in range(2)]
        # v_aug: per token-tile [128, HPG*65]; per head: 64 v-cols then a 1s col
        v_sb = [persist.tile([128, HPG * 65], BF16, tag=f"v{mt}", name=f"v{mt}")
                for mt in range(MT)]
        # concat head outputs, O^T layout [d, n], pair-major
        o_sb = [persist.tile([128, N], BF16, tag=f"oT{p}", name=f"oT{p}") for p in range(2)]

        # --- projections ----------------------------------------------------
        with tc.tile_pool(name="proj_ps", bufs=2, space="PSUM") as proj_ps:
            # q^T, k^T: [DG, N] as 2 pair-tiles of [128, N]
            for name, dst in (("q", qT_sb), ("k", kT_sb)):
                for p in range(2):
                    for nb in range(NB):
                        ps = proj_ps.tile([128, 512], F32, tag="projps", name="projps")
                        for kc in range(KC):
                            nc.tensor.matmul(
                                ps[:],
                                lhsT=w_sb[name, kc][:, p * 128:(p + 1) * 128],
                                rhs=xt_sb[kc][:, nb * 512:(nb + 1) * 512],
                                start=(kc == 0), stop=False)
                        nc.tensor.matmul(
                            ps[:],
                            lhsT=b_sb[name][:, p * 128:(p + 1) * 128],
                            rhs=ones[:, :512],
                            start=False, stop=True)
                        nc.vector.tensor_copy(
                            dst[p][:, nb * 512:(nb + 1) * 512], ps[:])
            # v: [N, DG] by token tiles, written into v_aug slots
            for mt in range(MT):
                ps = proj_ps.tile([128, DG], F32, tag="projps", name="projps2")
                for kc in range(KC):
                    nc.tensor.matmul(
                        ps[:],
                        lhsT=xt_sb[kc][:, mt * 128:(mt + 1) * 128],
                        rhs=w_sb["v", kc][:],
                        start=(kc == 0), stop=False)
                nc.tensor.matmul(
                    ps[:], lhsT=ones[:, :128], rhs=b_sb["v"][:],
                    start=False, stop=True)
                dst = v_sb[mt].rearrange("p (h c) -> p h c", c=65)
                nc.vector.tensor_copy(dst[:, :, 0:64],
                                      ps.rearrange("p (h c) -> p h c", c=64))
                nc.vector.memset(dst[:, :, 64:65], 1.0)

        # --- attention ------------------------------------------------------
        with (
            tc.tile_pool(name="s_ps", bufs=1, space="PSUM") as s_pool,
            tc.tile_pool(name="o_ps", bufs=4, space="PSUM") as o_pool,
            tc.tile_pool(name="e_sb", bufs=8) as e_pool,
            tc.tile_pool(name="attn_sm", bufs=8) as sm_pool,
        ):
            for h in range(HPG):
                p, hh = divmod(h, 2)
                qs = qT_sb[p][hh * 64:(hh + 1) * 64, :]
                ks = kT_sb[p][hh * 64:(hh + 1) * 64, :]
                e_tiles = []
                # scores^T tile-by-tile over m; exp to SBUF bf16
                for mt in range(MT):
                    s_ps = s_pool.tile([128, N], F32, tag="sps", name="sps")
                    for nb in range(NB):
                        nc.tensor.matmul(
                            s_ps[:, nb * 512:(nb + 1) * 512],
                            lhsT=qs[:, mt * 128:(mt + 1) * 128],
                            rhs=ks[:, nb * 512:(nb + 1) * 512],
                            start=True, stop=True)
                    e = e_pool.tile([128, N], BF16, tag="e", name="e")
                    nc.scalar.activation(e[:], s_ps[:],
                                         mybir.ActivationFunctionType.Exp,
                                         scale=SCALE)
                    e_tiles.append(e)
                # attn @ [v|1] accumulated over m; row 64 = colsum of E
                o_ps = [o_pool.tile([65, 512], F32, tag="ops", name="ops")
                        for _ in range(NB)]
                for mc in range(MT):
                    va = v_sb[mc].rearrange("p (h c) -> p h c", c=65)[:, h, :]
                    for nb in range(NB):
                        nc.tensor.matmul(
                            o_ps[nb][:],
                            lhsT=va,
                            rhs=e_tiles[mc][:, nb * 512:(nb + 1) * 512],
                            start=(mc == 0), stop=(mc == MT - 1))
                # normalize: O^T[d, n] * (1/colsum[n]) broadcast over d
                for nb in range(NB):
                    r = sm_pool.tile([1, 512], F32, tag="recip", name="recip")
                    nc.vector.reciprocal(r[:], o_ps[nb][64:65, :])
                    bc = sm_pool.tile([64, 512], F32, tag="bcast", name="bcast")
                    nc.gpsimd.partition_broadcast(bc[:], r[:])
                    nc.vector.tensor_mul(
                        o_sb[p][hh * 64:(hh + 1) * 64,
                                nb * 512:(nb + 1) * 512],
                        o_ps[nb][0:64, :], bc[:])

        # --- output projection (partial: this group's rows of Wo.T) ---------
        with (
            tc.tile_pool(name="out_ps", bufs=2, space="PSUM") as out_pool,
            tc.tile_pool(name="out_sb", bufs=2) as ostage,
        ):
            for ft in range(FT):
                stage = ostage.tile([128, N], F32, tag="ostage", name="ostage")
                for nb in range(NB):
                    ps = out_pool.tile([128, 512], F32, tag="outps", name="outps")
                    for pc in range(2):
                        nc.tensor.matmul(
                            ps[:],
                            lhsT=wo_sb[pc][:, ft * 128:(ft + 1) * 128],
                            rhs=o_sb[pc][:, nb * 512:(nb + 1) * 512],
                            start=(pc == 0), stop=(pc == 1))
                    nc.vector.tensor_copy(stage[:, nb * 512:(nb + 1) * 512],
                                          ps[:])
                nc.sync.dma_start(
                    out=outT.ap()[ft * 128:(ft + 1) * 128, :], in_=stage[:])


_CACHED_NC = None


def _get_nc():
    global _CACHED_NC
    if _CACHED_NC is None:
        _CACHED_NC = build_kernel()
    return _CACHED_NC


def make_in_maps(x, Wq, bq, Wk, bk, Wv, bv, Wo, bo):
    """Host-side shard/layout prep: per-core input dict."""
    x = np.asarray(x, dtype=np.float32)
    xT_b = [np.ascontiguousarray(x[b].T).astype(NPBF16) for b in range(B)]
    WqT = np.asarray(Wq, np.float32).T.astype(NPBF16)  # [DIM(feat), DIM(out)]
    WkT = np.asarray(Wk, np.float32).T.astype(NPBF16)
    WvT = np.asarray(Wv, np.float32).T.astype(NPBF16)
    WoT = np.asarray(Wo, np.float32).T.astype(NPBF16)  # rows: concat feats
    bq = np.asarray(bq, np.float32).astype(NPBF16)
    bk = np.asarray(bk, np.float32).astype(NPBF16)
    bv = np.asarray(bv, np.float32).astype(NPBF16)

    in_maps = []
    for c in range(N_CORES):
        b, g = divmod(c, GROUPS)
        sl = slice(g * DG, (g + 1) * DG)
        in_maps.append({
            "xT": xT_b[b],
            "wqT": np.ascontiguousarray(WqT[:, sl]),
            "wkT": np.ascontiguousarray(WkT[:, sl]),
            "wvT": np.ascontiguousarray(WvT[:, sl]),
            "woT": np.ascontiguousarray(WoT[sl, :]),
            "bq": bq[sl].reshape(1, DG),
            "bk": bk[sl].reshape(1, DG),
            "bv": bv[sl].reshape(1, DG),
        })
    return in_maps


def combine_outputs(results, bo):
    """Host-side unshard: sum group partials per batch, add bo."""
    bo = np.asarray(bo, np.float32)
    out = np.zeros((B, N, DIM), np.float32)
    for c in range(N_CORES):
        b = c // GROUPS
        out[b] += results[c]["outT"].T
    out += bo
    return out


def kernel(**inputs):
    nc = _get_nc()
    in_maps = make_in_maps(**{k: inputs[k] for k in
                              ("x", "Wq", "bq", "Wk", "bk", "Wv", "bv",
                               "Wo", "bo")})
    res = run_bass_kernel_spmd(nc, in_maps, list(range(N_CORES)))
    return combine_outputs(res.results, inputs["bo"])


if __name__ == "__main__":
    rng = np.random.default_rng(0)
    ins = {
        "x": rng.standard_normal((B, N, DIM), np.float32),
        "Wq": rng.standard_normal((DIM, DIM), np.float32) * 0.02,
        "bq": rng.standard_normal((DIM,), np.float32) * 0.02,
        "Wk": rng.standard_normal((DIM, DIM), np.float32) * 0.02,
        "bk": rng.standard_normal((DIM,), np.float32) * 0.02,
        "Wv": rng.standard_normal((DIM, DIM), np.float32) * 0.02,
        "bv": rng.standard_normal((DIM,), np.float32) * 0.02,
        "Wo": rng.standard_normal((DIM, DIM), np.float32) * 0.02,
        "bo": rng.standard_normal((DIM,), np.float32) * 0.02,
    }
    out = kernel(**ins)
    print("kernel output", out.shape, out.dtype, float(np.abs(out).mean()))


# revision 8
# speedup vs baseline: 1.4930x; 1.4930x over previous
"""Trainium2 Bass kernel for nn_MultiHeadAttention_5059471475068.

Reference computation (B=2, N=2048, DIM=1024, H=16 heads, d=64):
    q = x @ Wq.T + bq ; k = x @ Wk.T + bk ; v = x @ Wv.T + bv   (per-head split)
    scores[h,b,n,m] = (k[h,b,n,:] . q[h,b,m,:]) / sqrt(DIM)
    attn = softmax(scores, axis=m)
    out[h,b,n,:] = attn @ v ; out = concat_heads @ Wo.T + bo

Sharding: 8 cores = 2 batches x 4 head-groups (4 heads per core).
Each core computes its heads' q,k,v projections, attention, and a partial
output projection (its heads' columns of the concat times the matching rows
of Wo.T).  Host sums the 4 partials per batch and adds bo (the unshard step
for the tensor-parallel dimension).

On-chip layout: scores are computed transposed (S^T[m, n], partition = m) so
that E = exp(S^T) feeds the attn@v matmul directly as the moving operand
with contraction over m, with no transposes.  The softmax denominator
(column sum of E) is folded into the attn@v matmul by appending a
ones-column to v (stationary operand [v | 1], M=65): PSUM row 64 of the
attn@v output accumulates sum_m E[m, n] exactly in fp32.

Schedule: one software pipeline keyed to the ScalarE exp stream (the
second-busiest engine).  q/k of head-pair 0 are projected first (kc-outer,
DMA-paced); then per token-tile the emission interleaves, under head h's
S^T/exp stream: the v projection (h0), the pair-1 q/k projections (h0),
and head h-1's attn@v matmuls (h1..h3), so TensorE work hides under the
exp stream and ScalarE never starves.  PSUM budget: 4 banks S^T ping-pong,
4 banks attn@v accumulators / step-specific projection accumulators.
"""

import sys

if "/opt/trn_rl_repo" not in sys.path:
    sys.path.insert(0, "/opt/trn_rl_repo")

import numpy as np
import ml_dtypes

import concourse.bacc as bacc
import concourse.tile as tile
import concourse.mybir as mybir
from concourse.bass_utils import run_bass_kernel_spmd

BF16 = mybir.dt.bfloat16
F32 = mybir.dt.float32
NPBF16 = ml_dtypes.bfloat16

DIM = 1024
HEADS = 16
HEAD_DIM = 64
B, N = 2, 2048
SCALE = 1.0 / float(np.sqrt(np.float32(DIM)))

N_CORES = 8
GROUPS = 4             # head-groups (one per core within a batch)
HPG = HEADS // GROUPS  # heads per group = 4
DG = HPG * HEAD_DIM    # feature columns per group = 256

KC = DIM // 128        # contraction chunks over features = 8
MT = N // 128          # token tiles = 16
NB = N // 512          # 512-wide column blocks = 4
FT = DIM // 128        # output-feature tiles = 8
EXPW = 1024            # exp granularity (PSUM cols per S^T tile)
NH = N // EXPW         # halves per row-tile = 2


def build_kernel():
    """Build the per-core Bass program (identical on all cores; data differs)."""
    nc = bacc.Bacc("TRN2", target_bir_lowering=False, debug=False,
                   num_devices=N_CORES)

    xT = nc.dram_tensor("xT", [DIM, N], BF16, kind="ExternalInput")
    wqT = nc.dram_tensor("wqT", [DIM, DG], BF16, kind="ExternalInput")
    wkT = nc.dram_tensor("wkT", [DIM, DG], BF16, kind="ExternalInput")
    wvT = nc.dram_tensor("wvT", [DIM, DG], BF16, kind="ExternalInput")
    woT = nc.dram_tensor("woT", [DG, DIM], BF16, kind="ExternalInput")
    # q/k biases as per-pair columns [128, 2] f32 (partition = within-pair dim)
    bqc = nc.dram_tensor("bqc", [128, 2], F32, kind="ExternalInput")
    bkc = nc.dram_tensor("bkc", [128, 2], F32, kind="ExternalInput")
    bv = nc.dram_tensor("bv", [1, DG], BF16, kind="ExternalInput")
    outT = nc.dram_tensor("outT", [DIM, N], F32, kind="ExternalOutput")

    with tile.TileContext(nc) as tc:
        _body(nc, tc, xT, wqT, wkT, wvT, woT, bqc, bkc, bv, outT)

    nc.compile()
    return nc


def _body(nc, tc, xT, wqT, wkT, wvT, woT, bqc, bkc, bv, outT):
    from contextlib import ExitStack

    Exp = mybir.ActivationFunctionType.Exp

    with ExitStack() as ctx:
        persist = ctx.enter_context(tc.tile_pool(name="persist", bufs=1))
        e_pool = ctx.enter_context(tc.tile_pool(name="e_sb", bufs=36))
        sm_pool = ctx.enter_context(tc.tile_pool(name="attn_sm", bufs=8))
        xpool_cm = tc.tile_pool(name="xpool", bufs=1)
        xpool = xpool_cm.__enter__()

        # --- input loads: x/wq/wk interleaved per chunk (gates the ramp) ----
        xt_sb, wq_sb, wk_sb = [], [], []
        for kc in range(KC):
            t = xpool.tile([128, N], BF16, tag=f"xt{kc}", name=f"xt{kc}")
            nc.sync.dma_start(out=t[:], in_=xT.ap()[kc * 128:(kc + 1) * 128, :])
            xt_sb.append(t)
            t = xpool.tile([128, DG], BF16, tag=f"wq{kc}", name=f"wq{kc}")
            nc.sync.dma_start(out=t[:], in_=wqT.ap()[kc * 128:(kc + 1) * 128, :])
            wq_sb.append(t)
            t = xpool.tile([128, DG], BF16, tag=f"wk{kc}", name=f"wk{kc}")
            nc.sync.dma_start(out=t[:], in_=wkT.ap()[kc * 128:(kc + 1) * 128, :])
            wk_sb.append(t)
        bq_sb = persist.tile([128, 2], F32, tag="bq", name="bq")
        nc.sync.dma_start(out=bq_sb[:], in_=bqc.ap()[:, :])
        bk_sb = persist.tile([128, 2], F32, tag="bk", name="bk")
        nc.sync.dma_start(out=bk_sb[:], in_=bkc.ap()[:, :])
        wv_sb = []
        for kc in range(KC):
            t = xpool.tile([128, DG], BF16, tag=f"wv{kc}", name=f"wv{kc}")
            nc.sync.dma_start(out=t[:], in_=wvT.ap()[kc * 128:(kc + 1) * 128, :])
            wv_sb.append(t)
        bv_sb = xpool.tile([1, DG], BF16, tag="bv", name="bv")
        nc.sync.dma_start(out=bv_sb[:], in_=bv.ap()[:, :])
        wo_sb = []
        for pc in range(2):
            t = persist.tile([128, DIM], BF16, tag=f"wo{pc}", name=f"wo{pc}")
            nc.sync.dma_start(out=t[:], in_=woT.ap()[pc * 128:(pc + 1) * 128, :])
            wo_sb.append(t)
        ones = persist.tile([1, 512], BF16, tag="ones", name="ones")
        nc.vector.memset(ones[:], 1.0)
        # warm the ScalarE Exp table while DMAs stream in
        warm = persist.tile([1, 1], F32, tag="warm", name="warm")
        nc.scalar.activation(warm[:], ones[:, 0:1], Exp)

        # persistent activations
        qT_sb = [persist.tile([128, N], BF16, tag=f"qT{p}", name=f"qT{p}")
                 for p in range(2)]
        kT_sb = [persist.tile([128, N], BF16, tag=f"kT{p}", name=f"kT{p}")
                 for p in range(2)]
        v_sb = [persist.tile([128, HPG * 65], BF16, tag=f"v{mt}", name=f"v{mt}")
                for mt in range(MT)]
        o_sb = [persist.tile([128, N], BF16, tag=f"oT{p}", name=f"oT{p}")
                for p in range(2)]

        # --- phase 1: q/k projections for pair 0, kc-outer (DMA-paced) -----
        with tc.tile_pool(name="qk0_ps", bufs=1, space="PSUM") as qk0:
            qacc = [qk0.tile([128, 512], F32, tag=f"qacc{nb}",
                             name=f"qacc{nb}") for nb in range(NB)]
            kacc = [qk0.tile([128, 512], F32, tag=f"kacc{nb}",
                             name=f"kacc{nb}") for nb in range(NB)]
            for kc in range(KC):
                for nb in range(NB):
                    nc.tensor.matmul(
                        qacc[nb][:],
                        lhsT=wq_sb[kc][:, 0:128],
                        rhs=xt_sb[kc][:, nb * 512:(nb + 1) * 512],
                        start=(kc == 0), stop=(kc == KC - 1))
                    nc.tensor.matmul(
                        kacc[nb][:],
                        lhsT=wk_sb[kc][:, 0:128],
                        rhs=xt_sb[kc][:, nb * 512:(nb + 1) * 512],
                        start=(kc == 0), stop=(kc == KC - 1))
            for which, nb in (("q", 0), ("k", 0), ("k", 1), ("q", 1),
                              ("k", 2), ("k", 3), ("q", 2), ("q", 3)):
                acc, dst, bias = ((qacc, qT_sb, bq_sb) if which == "q"
                                  else (kacc, kT_sb, bk_sb))
                nc.vector.tensor_scalar_add(
                    dst[0][:, nb * 512:(nb + 1) * 512], acc[nb][:],
                    bias[:, 0:1])

        # --- attention pipeline ---------------------------------------------
        s_pool_cm = tc.tile_pool(name="s_ps", bufs=2, space="PSUM")
        s_pool = s_pool_cm.__enter__()

        e_tiles = {}   # (h, mt, half) -> tile
        o_ps = {}      # h -> [4 psum accumulators]

        def emit_s_exp(h, mt):
            """S^T tile + exp for (head, token-tile), NH halves."""
            p, hh = divmod(h, 2)
            qs = qT_sb[p][hh * 64:(hh + 1) * 64, :]
            ks = kT_sb[p][hh * 64:(hh + 1) * 64, :]
            for half in range(NH):
                s_ps = s_pool.tile([128, EXPW], F32, tag="sps", name="sps")
                for j in range(EXPW // 512):
                    c0 = half * EXPW + j * 512
                    nc.tensor.matmul(
                        s_ps[:, j * 512:(j + 1) * 512],
                        lhsT=qs[:, mt * 128:(mt + 1) * 128],
                        rhs=ks[:, c0:c0 + 512],
                        start=True, stop=True)
                e = e_pool.tile([128, EXPW], BF16, tag="e", name="e")
                nc.scalar.activation(e[:], s_ps[:], Exp, scale=SCALE)
                e_tiles[h, mt, half] = e

        def emit_av(h, mc, o_pool):
            """attn@[v|1] accumulation step for head h, m-chunk mc."""
            if mc == 0:
                o_ps[h] = [o_pool.tile([65, 512], F32, tag="ops",
                                       name="ops") for _ in range(NB)]
            va = v_sb[mc].rearrange("p (h c) -> p h c", c=65)[:, h, :]
            for nb in range(NB):
                e = e_tiles[h, mc, nb // 2]
                nc.tensor.matmul(
                    o_ps[h][nb][:],
                    lhsT=va,
                    rhs=e[:, (nb % 2) * 512:(nb % 2 + 1) * 512],
                    start=(mc == 0), stop=(mc == MT - 1))

        def emit_norm(h):
            """normalize O^T rows by the folded column-sums."""
            p, hh = divmod(h, 2)
            for nb in range(NB):
                r = sm_pool.tile([1, 512], F32, tag="recip", name="recip")
                nc.vector.reciprocal(r[:], o_ps[h][nb][64:65, :])
                bc = sm_pool.tile([64, 512], F32, tag="bcast", name="bcast")
                nc.gpsimd.partition_broadcast(bc[:], r[:])
                nc.vector.tensor_mul(
                    o_sb[p][hh * 64:(hh + 1) * 64, nb * 512:(nb + 1) * 512],
                    o_ps[h][nb][0:64, :], bc[:])
            for key in [k for k in e_tiles if k[0] == h]:
                del e_tiles[key]

        # --- step 2: head 0 S/exp + v projection + pair-1 q/k projections ---
        vps_cm = tc.tile_pool(name="vps", bufs=2, space="PSUM")
        vps = vps_cm.__enter__()
        p1ps_cm = tc.tile_pool(name="p1ps", bufs=2, space="PSUM")
        p1ps = p1ps_cm.__enter__()

        def emit_v(mt):
            ps = vps.tile([128, DG], F32, tag="vps", name="vpsn")
            for kc in range(KC):
                nc.tensor.matmul(
                    ps[:],
                    lhsT=xt_sb[kc][:, mt * 128:(mt + 1) * 128],
                    rhs=wv_sb[kc][:],
                    start=(kc == 0), stop=False)
            nc.tensor.matmul(
                ps[:], lhsT=ones[:, :128], rhs=bv_sb[:],
                start=False, stop=True)
            dst = v_sb[mt].rearrange("p (h c) -> p h c", c=65)
            nc.vector.tensor_copy(dst[:, :, 0:64],
                                  ps.rearrange("p (h c) -> p h c", c=64))
            nc.vector.memset(dst[:, :, 64:65], 1.0)

        def emit_p1_group(i):
            """one (name, nb) accumulation group of the pair-1 projections."""
            name, nb = divmod(i, NB)
            w, bias, dst = ((wq_sb, bq_sb, qT_sb) if name == 0
                            else (wk_sb, bk_sb, kT_sb))
            ps = p1ps.tile([128, 512], F32, tag="p1", name="p1")
            for kc in range(KC):
                nc.tensor.matmul(
                    ps[:],
                    lhsT=w[kc][:, 128:256],
                    rhs=xt_sb[kc][:, nb * 512:(nb + 1) * 512],
                    start=(kc == 0), stop=(kc == KC - 1))
            nc.vector.tensor_scalar_add(
                dst[1][:, nb * 512:(nb + 1) * 512], ps[:], bias[:, 1:2])

        for mt in range(MT):
            emit_v(mt)
            emit_s_exp(0, mt)
            if mt % 2 == 1:
                emit_p1_group(mt // 2)

        p1ps_cm.__exit__(None, None, None)
        vps_cm.__exit__(None, None, None)

        o_pool_cm = tc.tile_pool(name="o_ps", bufs=4, space="PSUM")
        o_pool = o_pool_cm.__enter__()

        # --- steps 3-4: heads 1-2 S/exp + previous head's attn@v ------------
        for h in (1, 2):
            for mt in range(MT):
                emit_s_exp(h, mt)
                emit_av(h - 1, mt, o_pool)
            emit_norm(h - 1)

        # --- step 5: head 3 S/exp + attn@v of heads 2 and 3 (2 chunks/mt) ---
        for mt in range(MT):
            emit_s_exp(3, mt)
            if mt < 8:
                emit_av(2, 2 * mt, o_pool)
                emit_av(2, 2 * mt + 1, o_pool)
                if mt == 7:
                    emit_norm(2)
            else:
                emit_av(3, 2 * (mt - 8), o_pool)
                emit_av(3, 2 * (mt - 8) + 1, o_pool)
        emit_norm(3)

        o_pool_cm.__exit__(None, None, None)
        s_pool_cm.__exit__(None, None, None)
        xpool_cm.__exit__(None, None, None)

        # --- output projection (partial: this group's rows of Wo.T) ---------
        with (
            tc.tile_pool(name="out_ps", bufs=4, space="PSUM") as out_pool,
            tc.tile_pool(name="out_sb", bufs=2) as ostage,
        ):
            for ft in range(FT):
                stage = ostage.tile([128, N], F32, tag="ostage", name="ostage")
                for nb in range(NB):
                    ps = out_pool.tile([128, 512], F32, tag="outps",
                                       name="outps")
                    for pc in range(2):
                        nc.tensor.matmul(
                            ps[:],
                            lhsT=wo_sb[pc][:, ft * 128:(ft + 1) * 128],
                            rhs=o_sb[pc][:, nb * 512:(nb + 1) * 512],
                            start=(pc == 0), stop=(pc == 1))
                    # both ScalarE and VectorE are idle by now; split drains
                    if nb % 2 == 0:
                        nc.scalar.copy(stage[:, nb * 512:(nb + 1) * 512],
                                       ps[:])
                    else:
                        nc.vector.tensor_copy(
                            stage[:, nb * 512:(nb + 1) * 512], ps[:])
                nc.sync.dma_start(
                    out=outT.ap()[ft * 128:(ft + 1) * 128, :], in_=stage[:])


_CACHED_NC = None


def _get_nc():
    global _CACHED_NC
    if _CACHED_NC is None:
        _CACHED_NC = build_kernel()
    return _CACHED_NC


def make_in_maps(x, Wq, bq, Wk, bk, Wv, bv, Wo, bo):
    """Host-side shard/layout prep: per-core input dict."""
    x = np.asarray(x, dtype=np.float32)
    xT_b = [np.ascontiguousarray(x[b].T).astype(NPBF16) for b in range(B)]
    WqT = np.asarray(Wq, np.float32).T.astype(NPBF16)  # [DIM(feat), DIM(out)]
    WkT = np.asarray(Wk, np.float32).T.astype(NPBF16)
    WvT = np.asarray(Wv, np.float32).T.astype(NPBF16)
    WoT = np.asarray(Wo, np.float32).T.astype(NPBF16)  # rows: concat feats
    bq = np.asarray(bq, np.float32)
    bk = np.asarray(bk, np.float32)
    bv16 = np.asarray(bv, np.float32).astype(NPBF16)

    in_maps = []
    for c in range(N_CORES):
        b, g = divmod(c, GROUPS)
        sl = slice(g * DG, (g + 1) * DG)
        in_maps.append({
            "xT": xT_b[b],
            "wqT": np.ascontiguousarray(WqT[:, sl]),
            "wkT": np.ascontiguousarray(WkT[:, sl]),
            "wvT": np.ascontiguousarray(WvT[:, sl]),
            "woT": np.ascontiguousarray(WoT[sl, :]),
            "bqc": np.ascontiguousarray(bq[sl].reshape(2, 128).T),
            "bkc": np.ascontiguousarray(bk[sl].reshape(2, 128).T),
            "bv": bv16[sl].reshape(1, DG),
        })
    return in_maps


def combine_outputs(results, bo):
    """Host-side unshard: sum group partials per batch, add bo."""
    bo = np.asarray(bo, np.float32)
    out = np.zeros((B, N, DIM), np.float32)
    for c in range(N_CORES):
        b = c // GROUPS
        out[b] += results[c]["outT"].T
    out += bo
    return out


def kernel(**inputs):
    nc = _get_nc()
    in_maps = make_in_maps(**{k: inputs[k] for k in
                              ("x", "Wq", "bq", "Wk", "bk", "Wv", "bv",
                               "Wo", "bo")})
    res = run_bass_kernel_spmd(nc, in_maps, list(range(N_CORES)))
    return combine_outputs(res.results, inputs["bo"])


if __name__ == "__main__":
    rng = np.random.default_rng(0)
    ins = {
        "x": rng.standard_normal((B, N, DIM), np.float32),
        "Wq": rng.standard_normal((DIM, DIM), np.float32) * 0.02,
        "bq": rng.standard_normal((DIM,), np.float32) * 0.02,
        "bk": rng.standard_normal((DIM,), np.float32) * 0.02,
        "Wk": rng.standard_normal((DIM, DIM), np.float32) * 0.02,
        "Wv": rng.standard_normal((DIM, DIM), np.float32) * 0.02,
        "bv": rng.standard_normal((DIM,), np.float32) * 0.02,
        "Wo": rng.standard_normal((DIM, DIM), np.float32) * 0.02,
        "bo": rng.standard_normal((DIM,), np.float32) * 0.02,
    }
    out = kernel(**ins)
    print("kernel output", out.shape, out.dtype, float(np.abs(out).mean()))


# revision 13
# speedup vs baseline: 1.5434x; 1.0337x over previous
"""Trainium2 Bass kernel for nn_MultiHeadAttention_5059471475068.

Reference computation (B=2, N=2048, DIM=1024, H=16 heads, d=64):
    q = x @ Wq.T + bq ; k = x @ Wk.T + bk ; v = x @ Wv.T + bv   (per-head split)
    scores[h,b,n,m] = (k[h,b,n,:] . q[h,b,m,:]) / sqrt(DIM)
    attn = softmax(scores, axis=m)
    out[h,b,n,:] = attn @ v ; out = concat_heads @ Wo.T + bo

Sharding: 8 cores = 2 batches x 4 head-groups (4 heads per core).
Each core computes its heads' q,k,v projections, attention, and a partial
output projection (its heads' columns of the concat times the matching rows
of Wo.T).  Host sums the 4 partials per batch and adds bo (the unshard step
for the tensor-parallel dimension).

On-chip layout: scores are computed transposed (S^T[m, n], partition = m) so
that E = exp(S^T) feeds the attn@v matmul directly as the moving operand
with contraction over m, with no transposes.  The softmax denominator
(column sum of E) is folded into the attn@v matmul by appending a
ones-column to v (stationary operand [v | 1], M=65): PSUM row 64 of the
attn@v output accumulates sum_m E[m, n] exactly in fp32.

Schedule: one software pipeline keyed to the ScalarE exp stream (the
second-busiest engine).  q/k of head-pair 0 are projected first (kc-outer,
DMA-paced); then per token-tile the emission interleaves, under head h's
S^T/exp stream: the v projection (h0), the pair-1 q/k projections (h0),
and head h-1's attn@v matmuls (h1..h3), so TensorE work hides under the
exp stream and ScalarE never starves.  PSUM budget: 4 banks S^T ping-pong,
4 banks attn@v accumulators / step-specific projection accumulators.
"""

import sys

if "/opt/trn_rl_repo" not in sys.path:
    sys.path.insert(0, "/opt/trn_rl_repo")

import numpy as np
import ml_dtypes

import concourse.bacc as bacc
import concourse.tile as tile
import concourse.mybir as mybir
from concourse.bass_utils import run_bass_kernel_spmd

BF16 = mybir.dt.bfloat16
F32 = mybir.dt.float32
FP8 = mybir.dt.float8e4
NPBF16 = ml_dtypes.bfloat16

# fp8e4m3 E/v with DoubleRow matmuls for attn@v (2 fp8 weights per PE cell,
# contraction 256/pass).  exp(S) is ~1.0-scale so e4m3 is well-conditioned,
# and numerator/denominator share the same quantized E so the softmax ratio
# error largely cancels.
USE_FP8_AV = True
VW = 80  # per-head v columns incl. ones col, padded to a 16-byte stride

DIM = 1024
HEADS = 16
HEAD_DIM = 64
B, N = 2, 2048
SCALE = 1.0 / float(np.sqrt(np.float32(DIM)))

N_CORES = 8
GROUPS = 4             # head-groups (one per core within a batch)
HPG = HEADS // GROUPS  # heads per group = 4
DG = HPG * HEAD_DIM    # feature columns per group = 256

KC = DIM // 128        # contraction chunks over features = 8
MT = N // 128          # token tiles = 16
NB = N // 512          # 512-wide column blocks = 4
FT = DIM // 128        # output-feature tiles = 8
EXPW = 1024            # exp granularity (PSUM cols per S^T tile)
NH = N // EXPW         # halves per row-tile = 2


def build_kernel(reps_loop=False):
    """Build the per-core Bass program (identical on all cores; data differs).

    reps_loop=True wraps the body in a data-driven repeat loop (input tensor
    "reps") used only by the timing harness; the graded path builds without.
    """
    nc = bacc.Bacc("TRN2", target_bir_lowering=False, debug=False,
                   num_devices=N_CORES)

    xT = nc.dram_tensor("xT", [DIM, N], BF16, kind="ExternalInput")
    wqT = nc.dram_tensor("wqT", [DIM, DG], BF16, kind="ExternalInput")
    wkT = nc.dram_tensor("wkT", [DIM, DG], BF16, kind="ExternalInput")
    wvT = nc.dram_tensor("wvT", [DIM, DG], BF16, kind="ExternalInput")
    woT = nc.dram_tensor("woT", [DG, DIM], BF16, kind="ExternalInput")
    # q/k biases as per-pair columns [128, 2] f32 (partition = within-pair dim)
    bqc = nc.dram_tensor("bqc", [128, 2], F32, kind="ExternalInput")
    bkc = nc.dram_tensor("bkc", [128, 2], F32, kind="ExternalInput")
    bv = nc.dram_tensor("bv", [1, DG], BF16, kind="ExternalInput")
    outT = nc.dram_tensor("outT", [DIM, N], BF16, kind="ExternalOutput")
    reps = (nc.dram_tensor("reps", [1, 1], mybir.dt.int32,
                           kind="ExternalInput") if reps_loop else None)

    with tile.TileContext(nc) as tc:
        if reps_loop:
            with tc.tile_pool(name="repsp", bufs=1) as rpool:
                rt = rpool.tile([1, 1], mybir.dt.int32, tag="reps",
                                name="repst")
                nc.sync.dma_start(out=rt[:], in_=reps.ap()[:, :])
                val = nc.sync.value_load(rt[0:1, 0:1], min_val=1,
                                         max_val=1 << 20)
                with tc.For_i(0, val, 1):
                    _body(nc, tc, xT, wqT, wkT, wvT, woT, bqc, bkc, bv, outT)
        else:
            _body(nc, tc, xT, wqT, wkT, wvT, woT, bqc, bkc, bv, outT)

    nc.compile()
    return nc


def _body(nc, tc, xT, wqT, wkT, wvT, woT, bqc, bkc, bv, outT):
    from contextlib import ExitStack

    Exp = mybir.ActivationFunctionType.Exp

    with ExitStack() as ctx:
        persist = ctx.enter_context(tc.tile_pool(name="persist", bufs=1))
        e_pool = ctx.enter_context(tc.tile_pool(name="e_sb", bufs=36))
        sm_pool = ctx.enter_context(tc.tile_pool(name="attn_sm", bufs=8))
        xpool_cm = tc.tile_pool(name="xpool", bufs=1)
        xpool = xpool_cm.__enter__()

        # --- input loads: x/wq/wk interleaved per chunk (gates the ramp) ----
        xt_sb, wq_sb, wk_sb = [], [], []
        for kc in range(KC):
            t = xpool.tile([128, N], BF16, tag=f"xt{kc}", name=f"xt{kc}")
            nc.sync.dma_start(out=t[:], in_=xT.ap()[kc * 128:(kc + 1) * 128, :])
            xt_sb.append(t)
            t = xpool.tile([128, DG], BF16, tag=f"wq{kc}", name=f"wq{kc}")
            nc.sync.dma_start(out=t[:], in_=wqT.ap()[kc * 128:(kc + 1) * 128, :])
            wq_sb.append(t)
            t = xpool.tile([128, DG], BF16, tag=f"wk{kc}", name=f"wk{kc}")
            nc.sync.dma_start(out=t[:], in_=wkT.ap()[kc * 128:(kc + 1) * 128, :])
            wk_sb.append(t)
        bq_sb = persist.tile([128, 2], F32, tag="bq", name="bq")
        nc.sync.dma_start(out=bq_sb[:], in_=bqc.ap()[:, :])
        bk_sb = persist.tile([128, 2], F32, tag="bk", name="bk")
        nc.sync.dma_start(out=bk_sb[:], in_=bkc.ap()[:, :])
        wv_sb = []
        for kc in range(KC):
            t = xpool.tile([128, DG], BF16, tag=f"wv{kc}", name=f"wv{kc}")
            nc.sync.dma_start(out=t[:], in_=wvT.ap()[kc * 128:(kc + 1) * 128, :])
            wv_sb.append(t)
        bv_sb = xpool.tile([1, DG], BF16, tag="bv", name="bv")
        nc.sync.dma_start(out=bv_sb[:], in_=bv.ap()[:, :])
        wo_sb = []
        for pc in range(2):
            t = persist.tile([128, DIM], BF16, tag=f"wo{pc}", name=f"wo{pc}")
            nc.sync.dma_start(out=t[:], in_=woT.ap()[pc * 128:(pc + 1) * 128, :])
            wo_sb.append(t)
        ones = persist.tile([1, 512], BF16, tag="ones", name="ones")
        nc.vector.memset(ones[:], 1.0)
        # warm the ScalarE Exp table while DMAs stream in
        warm = persist.tile([1, 1], F32, tag="warm", name="warm")
        nc.scalar.activation(warm[:], ones[:, 0:1], Exp)

        # persistent activations
        qT_sb = [persist.tile([128, N], BF16, tag=f"qT{p}", name=f"qT{p}")
                 for p in range(2)]
        kT_sb = [persist.tile([128, N], BF16, tag=f"kT{p}", name=f"kT{p}")
                 for p in range(2)]
        if USE_FP8_AV:
            # paired token-tiles for DoubleRow: [128, (2, HPG, VW)] fp8
            v_sb = [persist.tile([128, 2 * HPG * VW], FP8, tag=f"v{mp}",
                                 name=f"v{mp}") for mp in range(MT // 2)]
        else:
            v_sb = [persist.tile([128, HPG * 65], BF16, tag=f"v{mt}",
                                 name=f"v{mt}") for mt in range(MT)]
        o_sb = [persist.tile([128, N], BF16, tag=f"oT{p}", name=f"oT{p}")
                for p in range(2)]

        # --- phase 1: q/k projections for pair 0, kc-outer (DMA-paced) -----
        with tc.tile_pool(name="qk0_ps", bufs=1, space="PSUM") as qk0:
            qacc = [qk0.tile([128, 512], F32, tag=f"qacc{nb}",
                             name=f"qacc{nb}") for nb in range(NB)]
            kacc = [qk0.tile([128, 512], F32, tag=f"kacc{nb}",
                             name=f"kacc{nb}") for nb in range(NB)]
            for kc in range(KC):
                for nb in range(NB):
                    nc.tensor.matmul(
                        qacc[nb][:],
                        lhsT=wq_sb[kc][:, 0:128],
                        rhs=xt_sb[kc][:, nb * 512:(nb + 1) * 512],
                        start=(kc == 0), stop=(kc == KC - 1))
                    nc.tensor.matmul(
                        kacc[nb][:],
                        lhsT=wk_sb[kc][:, 0:128],
                        rhs=xt_sb[kc][:, nb * 512:(nb + 1) * 512],
                        start=(kc == 0), stop=(kc == KC - 1))
            Ident = mybir.ActivationFunctionType.Identity
            for i, (which, nb) in enumerate(
                    (("q", 0), ("k", 0), ("k", 1), ("q", 1),
                     ("k", 2), ("k", 3), ("q", 2), ("q", 3))):
                acc, dst, bias = ((qacc, qT_sb, bq_sb) if which == "q"
                                  else (kacc, kT_sb, bk_sb))
                if i % 2 == 0:
                    nc.vector.tensor_scalar_add(
                        dst[0][:, nb * 512:(nb + 1) * 512], acc[nb][:],
                        bias[:, 0:1])
                else:
                    nc.scalar.activation(
                        dst[0][:, nb * 512:(nb + 1) * 512], acc[nb][:],
                        Ident, bias=bias[:, 0:1])

        # --- attention pipeline ---------------------------------------------
        s_pool_cm = tc.tile_pool(name="s_ps", bufs=2, space="PSUM")
        s_pool = s_pool_cm.__enter__()

        e_tiles = {}   # (h, mt, half) -> tile
        o_ps = {}      # h -> [4 psum accumulators]

        def emit_s_exp(h, mt):
            """S^T tile + exp for (head, token-tile), NH halves."""
            p, hh = divmod(h, 2)
            qs = qT_sb[p][hh * 64:(hh + 1) * 64, :]
            ks = kT_sb[p][hh * 64:(hh + 1) * 64, :]
            for half in range(NH):
                s_ps = s_pool.tile([128, EXPW], F32, tag="sps", name="sps")
                for j in range(EXPW // 512):
                    c0 = half * EXPW + j * 512
                    nc.tensor.matmul(
                        s_ps[:, j * 512:(j + 1) * 512],
                        lhsT=qs[:, mt * 128:(mt + 1) * 128],
                        rhs=ks[:, c0:c0 + 512],
                        start=True, stop=True)
                if USE_FP8_AV:
                    if mt % 2 == 0:
                        e_tiles[h, mt // 2, half] = e_pool.tile(
                            [128, 2 * EXPW], FP8, tag="e", name="e")
                    ep = e_tiles[h, mt // 2, half]
                    dst = ep.rearrange("p (two n) -> p two n",
                                       two=2)[:, mt % 2]
                    nc.scalar.activation(dst, s_ps[:], Exp, scale=SCALE)
                else:
                    e = e_pool.tile([128, EXPW], BF16, tag="e", name="e")
                    nc.scalar.activation(e[:], s_ps[:], Exp, scale=SCALE)
                    e_tiles[h, mt, half] = e

        def emit_av(h, mc, o_pool):
            """attn@[v|1] accumulation step for head h, m-chunk mc.

            fp8 path: mc indexes 256-row DoubleRow chunks (0..MT//2-1);
            bf16 path: mc indexes 128-row chunks (0..MT-1).
            """
            if mc == 0:
                o_ps[h] = [o_pool.tile([65, 512], F32, tag="ops",
                                       name="ops") for _ in range(NB)]
            if USE_FP8_AV:
                va = v_sb[mc].rearrange("p (two h c) -> p two h c",
                                        two=2, c=VW)[:, :, h, 0:65]
                for nb in range(NB):
                    ep = e_tiles[h, mc, nb // 2].rearrange(
                        "p (two n) -> p two n", two=2)
                    nc.tensor.matmul(
                        o_ps[h][nb][:],
                        lhsT=va,
                        rhs=ep[:, :, (nb % 2) * 512:(nb % 2 + 1) * 512],
                        start=(mc == 0), stop=(mc == MT // 2 - 1),
                        perf_mode=mybir.MatmulPerfMode.DoubleRow)
            else:
                va = v_sb[mc].rearrange("p (h c) -> p h c", c=65)[:, h, :]
                for nb in range(NB):
                    e = e_tiles[h, mc, nb // 2]
                    nc.tensor.matmul(
                        o_ps[h][nb][:],
                        lhsT=va,
                        rhs=e[:, (nb % 2) * 512:(nb % 2 + 1) * 512],
                        start=(mc == 0), stop=(mc == MT - 1))

        def emit_norm(h, nbs=None):
            """normalize O^T rows by the folded column-sums."""
            p, hh = divmod(h, 2)
            for nb in (range(NB) if nbs is None else nbs):
                r = sm_pool.tile([1, 512], F32, tag="recip", name="recip")
                nc.vector.reciprocal(r[:], o_ps[h][nb][64:65, :])
                bc = sm_pool.tile([64, 512], F32, tag="bcast", name="bcast")
                nc.gpsimd.partition_broadcast(bc[:], r[:])
                nc.vector.tensor_mul(
                    o_sb[p][hh * 64:(hh + 1) * 64, nb * 512:(nb + 1) * 512],
                    o_ps[h][nb][0:64, :], bc[:])
            if nbs is None or list(nbs)[-1] == NB - 1:
                for key in [k for k in e_tiles if k[0] == h]:
                    del e_tiles[key]

        # --- step 2: head 0 S/exp + v projection + pair-1 q/k projections ---
        vps_cm = tc.tile_pool(name="vps", bufs=2, space="PSUM")
        vps = vps_cm.__enter__()
        p1ps_cm = tc.tile_pool(name="p1ps", bufs=2, space="PSUM")
        p1ps = p1ps_cm.__enter__()

        def emit_v(mt):
            ps = vps.tile([128, DG], F32, tag="vps", name="vpsn")
            for kc in range(KC):
                nc.tensor.matmul(
                    ps[:],
                    lhsT=xt_sb[kc][:, mt * 128:(mt + 1) * 128],
                    rhs=wv_sb[kc][:],
                    start=(kc == 0), stop=False)
            nc.tensor.matmul(
                ps[:], lhsT=ones[:, :128], rhs=bv_sb[:],
                start=False, stop=True)
            if USE_FP8_AV:
                dst = v_sb[mt // 2].rearrange(
                    "p (two h c) -> p two h c", two=2, c=VW)[:, mt % 2]
            else:
                dst = v_sb[mt].rearrange("p (h c) -> p h c", c=65)
            nc.vector.tensor_copy(dst[:, :, 0:64],
                                  ps.rearrange("p (h c) -> p h c", c=64))
            nc.vector.memset(dst[:, :, 64:65], 1.0)

        def emit_p1_group(i):
            """one (name, nb) accumulation group of the pair-1 projections."""
            name, nb = divmod(i, NB)
            w, bias, dst = ((wq_sb, bq_sb, qT_sb) if name == 0
                            else (wk_sb, bk_sb, kT_sb))
            ps = p1ps.tile([128, 512], F32, tag="p1", name="p1")
            for kc in range(KC):
                nc.tensor.matmul(
                    ps[:],
                    lhsT=w[kc][:, 128:256],
                    rhs=xt_sb[kc][:, nb * 512:(nb + 1) * 512],
                    start=(kc == 0), stop=(kc == KC - 1))
            nc.vector.tensor_scalar_add(
                dst[1][:, nb * 512:(nb + 1) * 512], ps[:], bias[:, 1:2])

        for mt in range(MT):
            emit_v(mt)
            emit_s_exp(0, mt)
            if mt % 2 == 1:
                emit_p1_group(mt // 2)

        p1ps_cm.__exit__(None, None, None)
        vps_cm.__exit__(None, None, None)

        o_pool_cm = tc.tile_pool(name="o_ps", bufs=4, space="PSUM")
        o_pool = o_pool_cm.__enter__()

        # --- steps 3-4: heads 1-2 S/exp + previous head's attn@v ------------
        for h in (1, 2):
            for mt in range(MT):
                emit_s_exp(h, mt)
                if USE_FP8_AV:
                    if mt % 2 == 1:
                        emit_av(h - 1, mt // 2, o_pool)
                else:
                    emit_av(h - 1, mt, o_pool)
            emit_norm(h - 1)

        # --- step 5: head 3 S/exp + attn@v of heads 2 and 3 -----------------
        for mt in range(MT):
            emit_s_exp(3, mt)
            if USE_FP8_AV:
                if mt < 8:
                    emit_av(2, mt, o_pool)
                    if mt == 7:
                        emit_norm(2)
                else:
                    emit_av(3, mt - 8, o_pool)
            else:
                if mt < 8:
                    emit_av(2, 2 * mt, o_pool)
                    emit_av(2, 2 * mt + 1, o_pool)
                    if mt == 7:
                        emit_norm(2)
                else:
                    emit_av(3, 2 * (mt - 8), o_pool)
                    emit_av(3, 2 * (mt - 8) + 1, o_pool)
        emit_norm(3)

        o_pool_cm.__exit__(None, None, None)
        s_pool_cm.__exit__(None, None, None)
        xpool_cm.__exit__(None, None, None)

        # --- output projection (partial: this group's rows of Wo.T) ---------
        # nb-outer so norm(3, nb) -> matmuls -> drains -> DMA pipeline per
        # column block; output in bf16 to halve the tail DMA.
        with (
            tc.tile_pool(name="out_ps", bufs=8, space="PSUM") as out_pool,
            tc.tile_pool(name="out_sb", bufs=8) as ostage,
        ):
            for nb in range(NB):
                for ft in range(FT):
                    ps = out_pool.tile([128, 512], F32, tag="outps",
                                       name="outps")
                    for pc in range(2):
                        nc.tensor.matmul(
                            ps[:],
                            lhsT=wo_sb[pc][:, ft * 128:(ft + 1) * 128],
                            rhs=o_sb[pc][:, nb * 512:(nb + 1) * 512],
                            start=(pc == 0), stop=(pc == 1))
                    stage = ostage.tile([128, 512], BF16, tag="ostage",
                                        name="ostage")
                    # both ScalarE and VectorE are idle by now; split drains
                    if ft % 2 == 0:
                        nc.scalar.copy(stage[:], ps[:])
                    else:
                        nc.vector.tensor_copy(stage[:], ps[:])
                    nc.sync.dma_start(
                        out=outT.ap()[ft * 128:(ft + 1) * 128,
                                      nb * 512:(nb + 1) * 512],
                        in_=stage[:])


_CACHED_NC = None


def _get_nc():
    global _CACHED_NC
    if _CACHED_NC is None:
        _CACHED_NC = build_kernel()
    return _CACHED_NC


def make_in_maps(x, Wq, bq, Wk, bk, Wv, bv, Wo, bo):
    """Host-side shard/layout prep: per-core input dict."""
    x = np.asarray(x, dtype=np.float32)
    xT_b = [np.ascontiguousarray(x[b].T).astype(NPBF16) for b in range(B)]
    WqT = np.asarray(Wq, np.float32).T.astype(NPBF16)  # [DIM(feat), DIM(out)]
    WkT = np.asarray(Wk, np.float32).T.astype(NPBF16)
    WvT = np.asarray(Wv, np.float32).T.astype(NPBF16)
    WoT = np.asarray(Wo, np.float32).T.astype(NPBF16)  # rows: concat feats
    bq = np.asarray(bq, np.float32)
    bk = np.asarray(bk, np.float32)
    bv16 = np.asarray(bv, np.float32).astype(NPBF16)

    in_maps = []
    for c in range(N_CORES):
        b, g = divmod(c, GROUPS)
        sl = slice(g * DG, (g + 1) * DG)
        in_maps.append({
            "xT": xT_b[b],
            "wqT": np.ascontiguousarray(WqT[:, sl]),
            "wkT": np.ascontiguousarray(WkT[:, sl]),
            "wvT": np.ascontiguousarray(WvT[:, sl]),
            "woT": np.ascontiguousarray(WoT[sl, :]),
            "bqc": np.ascontiguousarray(bq[sl].reshape(2, 128).T),
            "bkc": np.ascontiguousarray(bk[sl].reshape(2, 128).T),
            "bv": bv16[sl].reshape(1, DG),
        })
    return in_maps


def combine_outputs(results, bo):
    """Host-side unshard: sum group partials per batch, add bo."""
    bo = np.asarray(bo, np.float32)
    out = np.zeros((B, N, DIM), np.float32)
    for c in range(N_CORES):
        b = c // GROUPS
        out[b] += results[c]["outT"].astype(np.float32).T
    out += bo
    return out


def kernel(**inputs):
    nc = _get_nc()
    in_maps = make_in_maps(**{k: inputs[k] for k in
                              ("x", "Wq", "bq", "Wk", "bk", "Wv", "bv",
                               "Wo", "bo")})
    res = run_bass_kernel_spmd(nc, in_maps, list(range(N_CORES)))
    return combine_outputs(res.results, inputs["bo"])


if __name__ == "__main__":
    rng = np.random.default_rng(0)
    ins = {
        "x": rng.standard_normal((B, N, DIM), np.float32),
        "Wq": rng.standard_normal((DIM, DIM), np.float32) * 0.02,
        "bq": rng.standard_normal((DIM,), np.float32) * 0.02,
        "bk": rng.standard_normal((DIM,), np.float32) * 0.02,
        "Wk": rng.standard_normal((DIM, DIM), np.float32) * 0.02,
        "Wv": rng.standard_normal((DIM, DIM), np.float32) * 0.02,
        "bv": rng.standard_normal((DIM,), np.float32) * 0.02,
        "Wo": rng.standard_normal((DIM, DIM), np.float32) * 0.02,
        "bo": rng.standard_normal((DIM,), np.float32) * 0.02,
    }
    out = kernel(**ins)
    print("kernel output", out.shape, out.dtype, float(np.abs(out).mean()))


# revision 15
# speedup vs baseline: 1.5649x; 1.0139x over previous
"""Trainium2 Bass kernel for nn_MultiHeadAttention_5059471475068.

Reference computation (B=2, N=2048, DIM=1024, H=16 heads, d=64):
    q = x @ Wq.T + bq ; k = x @ Wk.T + bk ; v = x @ Wv.T + bv   (per-head split)
    scores[h,b,n,m] = (k[h,b,n,:] . q[h,b,m,:]) / sqrt(DIM)
    attn = softmax(scores, axis=m)
    out[h,b,n,:] = attn @ v ; out = concat_heads @ Wo.T + bo

Sharding: 8 cores = 2 batches x 4 head-groups (4 heads per core).
Each core computes its heads' q,k,v projections, attention, and a partial
output projection (its heads' columns of the concat times the matching rows
of Wo.T).  Host sums the 4 partials per batch and adds bo (the unshard step
for the tensor-parallel dimension).

On-chip layout: scores are computed transposed (S^T[m, n], partition = m) so
that E = exp(S^T) feeds the attn@v matmul directly as the moving operand
with contraction over m, with no transposes.  The softmax denominator
(column sum of E) is folded into the attn@v matmul by appending a
ones-column to v (stationary operand [v | 1], M=65): PSUM row 64 of the
attn@v output accumulates sum_m E[m, n] exactly in fp32.

Schedule: one software pipeline keyed to the ScalarE exp stream (the
second-busiest engine).  q/k of head-pair 0 are projected first (kc-outer,
DMA-paced); then per token-tile the emission interleaves, under head h's
S^T/exp stream: the v projection (h0), the pair-1 q/k projections (h0),
and head h-1's attn@v matmuls (h1..h3), so TensorE work hides under the
exp stream and ScalarE never starves.  PSUM budget: 4 banks S^T ping-pong,
4 banks attn@v accumulators / step-specific projection accumulators.
"""

import sys

if "/opt/trn_rl_repo" not in sys.path:
    sys.path.insert(0, "/opt/trn_rl_repo")

import numpy as np
import ml_dtypes

import concourse.bacc as bacc
import concourse.tile as tile
import concourse.mybir as mybir
from concourse.bass_utils import run_bass_kernel_spmd

BF16 = mybir.dt.bfloat16
F32 = mybir.dt.float32
FP8 = mybir.dt.float8e4
NPBF16 = ml_dtypes.bfloat16

# fp8e4m3 E/v with DoubleRow matmuls for attn@v (2 fp8 weights per PE cell,
# contraction 256/pass).  exp(S) is ~1.0-scale so e4m3 is well-conditioned,
# and numerator/denominator share the same quantized E so the softmax ratio
# error largely cancels.
USE_FP8_AV = False
VW = 80  # per-head v columns incl. ones col, padded to a 16-byte stride

DIM = 1024
HEADS = 16
HEAD_DIM = 64
B, N = 2, 2048
SCALE = 1.0 / float(np.sqrt(np.float32(DIM)))

N_CORES = 8
GROUPS = 4             # head-groups (one per core within a batch)
HPG = HEADS // GROUPS  # heads per group = 4
DG = HPG * HEAD_DIM    # feature columns per group = 256

KC = DIM // 128        # contraction chunks over features = 8
MT = N // 128          # token tiles = 16
NB = N // 512          # 512-wide column blocks = 4
FT = DIM // 128        # output-feature tiles = 8
EXPW = 1024            # exp granularity (PSUM cols per S^T tile)
NH = N // EXPW         # halves per row-tile = 2


def build_kernel(reps_loop=False):
    """Build the per-core Bass program (identical on all cores; data differs).

    reps_loop=True wraps the body in a data-driven repeat loop (input tensor
    "reps") used only by the timing harness; the graded path builds without.
    """
    nc = bacc.Bacc("TRN2", target_bir_lowering=False, debug=False,
                   num_devices=N_CORES)

    xT = nc.dram_tensor("xT", [DIM, N], BF16, kind="ExternalInput")
    wqT = nc.dram_tensor("wqT", [DIM, DG], BF16, kind="ExternalInput")
    wkT = nc.dram_tensor("wkT", [DIM, DG], BF16, kind="ExternalInput")
    wvT = nc.dram_tensor("wvT", [DIM, DG], BF16, kind="ExternalInput")
    woT = nc.dram_tensor("woT", [DG, DIM], BF16, kind="ExternalInput")
    # q/k biases as per-pair columns [128, 2] f32 (partition = within-pair dim)
    bqc = nc.dram_tensor("bqc", [128, 2], F32, kind="ExternalInput")
    bkc = nc.dram_tensor("bkc", [128, 2], F32, kind="ExternalInput")
    bv = nc.dram_tensor("bv", [1, DG], BF16, kind="ExternalInput")
    outT = nc.dram_tensor("outT", [DIM, N], BF16, kind="ExternalOutput")
    reps = (nc.dram_tensor("reps", [1, 1], mybir.dt.int32,
                           kind="ExternalInput") if reps_loop else None)

    with tile.TileContext(nc) as tc:
        if reps_loop:
            with tc.tile_pool(name="repsp", bufs=1) as rpool:
                rt = rpool.tile([1, 1], mybir.dt.int32, tag="reps",
                                name="repst")
                nc.sync.dma_start(out=rt[:], in_=reps.ap()[:, :])
                val = nc.sync.value_load(rt[0:1, 0:1], min_val=1,
                                         max_val=1 << 20)
                with tc.For_i(0, val, 1):
                    _body(nc, tc, xT, wqT, wkT, wvT, woT, bqc, bkc, bv, outT)
        else:
            _body(nc, tc, xT, wqT, wkT, wvT, woT, bqc, bkc, bv, outT)

    nc.compile()
    return nc


def _body(nc, tc, xT, wqT, wkT, wvT, woT, bqc, bkc, bv, outT):
    from contextlib import ExitStack

    Exp = mybir.ActivationFunctionType.Exp

    with ExitStack() as ctx:
        persist = ctx.enter_context(tc.tile_pool(name="persist", bufs=1))
        e_pool = ctx.enter_context(tc.tile_pool(name="e_sb", bufs=36))
        sm_pool = ctx.enter_context(tc.tile_pool(name="attn_sm", bufs=8))
        xpool_cm = tc.tile_pool(name="xpool", bufs=1)
        xpool = xpool_cm.__enter__()

        # --- input loads: x/wq/wk interleaved per chunk (gates the ramp) ----
        xt_sb, wq_sb, wk_sb = [], [], []
        for kc in range(KC):
            t = xpool.tile([128, N], BF16, tag=f"xt{kc}", name=f"xt{kc}")
            nc.sync.dma_start(out=t[:], in_=xT.ap()[kc * 128:(kc + 1) * 128, :])
            xt_sb.append(t)
            t = xpool.tile([128, DG], BF16, tag=f"wq{kc}", name=f"wq{kc}")
            nc.sync.dma_start(out=t[:], in_=wqT.ap()[kc * 128:(kc + 1) * 128, :])
            wq_sb.append(t)
            t = xpool.tile([128, DG], BF16, tag=f"wk{kc}", name=f"wk{kc}")
            nc.sync.dma_start(out=t[:], in_=wkT.ap()[kc * 128:(kc + 1) * 128, :])
            wk_sb.append(t)
        bq_sb = persist.tile([128, 2], F32, tag="bq", name="bq")
        nc.sync.dma_start(out=bq_sb[:], in_=bqc.ap()[:, :])
        bk_sb = persist.tile([128, 2], F32, tag="bk", name="bk")
        nc.sync.dma_start(out=bk_sb[:], in_=bkc.ap()[:, :])
        wv_sb = []
        for kc in range(KC):
            t = xpool.tile([128, DG], BF16, tag=f"wv{kc}", name=f"wv{kc}")
            nc.sync.dma_start(out=t[:], in_=wvT.ap()[kc * 128:(kc + 1) * 128, :])
            wv_sb.append(t)
        bv_sb = xpool.tile([1, DG], BF16, tag="bv", name="bv")
        nc.sync.dma_start(out=bv_sb[:], in_=bv.ap()[:, :])
        wo_sb = []
        for pc in range(2):
            t = persist.tile([128, DIM], BF16, tag=f"wo{pc}", name=f"wo{pc}")
            nc.sync.dma_start(out=t[:], in_=woT.ap()[pc * 128:(pc + 1) * 128, :])
            wo_sb.append(t)
        ones = persist.tile([1, 512], BF16, tag="ones", name="ones")
        nc.vector.memset(ones[:], 1.0)
        # warm the ScalarE Exp table while DMAs stream in
        warm = persist.tile([1, 1], F32, tag="warm", name="warm")
        nc.scalar.activation(warm[:], ones[:, 0:1], Exp)

        # persistent activations
        qT_sb = [persist.tile([128, N], BF16, tag=f"qT{p}", name=f"qT{p}")
                 for p in range(2)]
        kT_sb = [persist.tile([128, N], BF16, tag=f"kT{p}", name=f"kT{p}")
                 for p in range(2)]
        if USE_FP8_AV:
            # paired token-tiles for DoubleRow: [128, (2, HPG, VW)] fp8
            v_sb = [persist.tile([128, 2 * HPG * VW], FP8, tag=f"v{mp}",
                                 name=f"v{mp}") for mp in range(MT // 2)]
        else:
            v_sb = [persist.tile([128, HPG * 65], BF16, tag=f"v{mt}",
                                 name=f"v{mt}") for mt in range(MT)]
        o_sb = [persist.tile([128, N], BF16, tag=f"oT{p}", name=f"oT{p}")
                for p in range(2)]

        # --- phase 1: q/k projections for pair 0, kc-outer (DMA-paced) -----
        with tc.tile_pool(name="qk0_ps", bufs=1, space="PSUM") as qk0:
            qacc = [qk0.tile([128, 512], F32, tag=f"qacc{nb}",
                             name=f"qacc{nb}") for nb in range(NB)]
            kacc = [qk0.tile([128, 512], F32, tag=f"kacc{nb}",
                             name=f"kacc{nb}") for nb in range(NB)]
            for kc in range(KC):
                for nb in range(NB):
                    nc.tensor.matmul(
                        qacc[nb][:],
                        lhsT=wq_sb[kc][:, 0:128],
                        rhs=xt_sb[kc][:, nb * 512:(nb + 1) * 512],
                        start=(kc == 0), stop=(kc == KC - 1))
                    nc.tensor.matmul(
                        kacc[nb][:],
                        lhsT=wk_sb[kc][:, 0:128],
                        rhs=xt_sb[kc][:, nb * 512:(nb + 1) * 512],
                        start=(kc == 0), stop=(kc == KC - 1))
            Ident = mybir.ActivationFunctionType.Identity
            for i, (which, nb) in enumerate(
                    (("q", 0), ("k", 0), ("k", 1), ("q", 1),
                     ("k", 2), ("k", 3), ("q", 2), ("q", 3))):
                acc, dst, bias = ((qacc, qT_sb, bq_sb) if which == "q"
                                  else (kacc, kT_sb, bk_sb))
                if i % 2 == 0:
                    nc.vector.tensor_scalar_add(
                        dst[0][:, nb * 512:(nb + 1) * 512], acc[nb][:],
                        bias[:, 0:1])
                else:
                    nc.scalar.activation(
                        dst[0][:, nb * 512:(nb + 1) * 512], acc[nb][:],
                        Ident, bias=bias[:, 0:1])

        # --- attention pipeline ---------------------------------------------
        s_pool_cm = tc.tile_pool(name="s_ps", bufs=2, space="PSUM")
        s_pool = s_pool_cm.__enter__()

        e_tiles = {}   # (h, mt, half) -> tile
        o_ps = {}      # h -> [4 psum accumulators]

        def emit_s_exp(h, mt):
            """S^T tile + exp for (head, token-tile), NH halves."""
            p, hh = divmod(h, 2)
            qs = qT_sb[p][hh * 64:(hh + 1) * 64, :]
            ks = kT_sb[p][hh * 64:(hh + 1) * 64, :]
            for half in range(NH):
                s_ps = s_pool.tile([128, EXPW], F32, tag="sps", name="sps")
                for j in range(EXPW // 512):
                    c0 = half * EXPW + j * 512
                    nc.tensor.matmul(
                        s_ps[:, j * 512:(j + 1) * 512],
                        lhsT=qs[:, mt * 128:(mt + 1) * 128],
                        rhs=ks[:, c0:c0 + 512],
                        start=True, stop=True)
                if USE_FP8_AV:
                    if mt % 2 == 0:
                        e_tiles[h, mt // 2, half] = e_pool.tile(
                            [128, 2 * EXPW], FP8, tag="e", name="e")
                    ep = e_tiles[h, mt // 2, half]
                    dst = ep.rearrange("p (two n) -> p two n",
                                       two=2)[:, mt % 2]
                    nc.scalar.activation(dst, s_ps[:], Exp, scale=SCALE)
                else:
                    e = e_pool.tile([128, EXPW], BF16, tag="e", name="e")
                    nc.scalar.activation(e[:], s_ps[:], Exp, scale=SCALE)
                    e_tiles[h, mt, half] = e

        def emit_av(h, mc, o_pool):
            """attn@[v|1] accumulation step for head h, m-chunk mc.

            fp8 path: mc indexes 256-row DoubleRow chunks (0..MT//2-1);
            bf16 path: mc indexes 128-row chunks (0..MT-1).
            """
            if mc == 0:
                o_ps[h] = [o_pool.tile([65, 512], F32, tag="ops",
                                       name="ops") for _ in range(NB)]
            if USE_FP8_AV:
                va = v_sb[mc].rearrange("p (two h c) -> p two h c",
                                        two=2, c=VW)[:, :, h, 0:65]
                for nb in range(NB):
                    ep = e_tiles[h, mc, nb // 2].rearrange(
                        "p (two n) -> p two n", two=2)
                    nc.tensor.matmul(
                        o_ps[h][nb][:],
                        lhsT=va,
                        rhs=ep[:, :, (nb % 2) * 512:(nb % 2 + 1) * 512],
                        start=(mc == 0), stop=(mc == MT // 2 - 1),
                        perf_mode=mybir.MatmulPerfMode.DoubleRow)
            else:
                va = v_sb[mc].rearrange("p (h c) -> p h c", c=65)[:, h, :]
                for nb in range(NB):
                    e = e_tiles[h, mc, nb // 2]
                    nc.tensor.matmul(
                        o_ps[h][nb][:],
                        lhsT=va,
                        rhs=e[:, (nb % 2) * 512:(nb % 2 + 1) * 512],
                        start=(mc == 0), stop=(mc == MT - 1))

        def emit_norm(h, nbs=None):
            """normalize O^T rows by the folded column-sums.

            Stage-major emission (recips, then broadcasts, then multiplies)
            so the three engines pipeline across the column blocks.
            """
            p, hh = divmod(h, 2)
            nbs = list(range(NB) if nbs is None else nbs)
            rs, bcs = {}, {}
            for nb in nbs:
                rs[nb] = sm_pool.tile([1, 512], F32, tag="recip",
                                      name="recip")
                nc.vector.reciprocal(rs[nb][:], o_ps[h][nb][64:65, :])
            for nb in nbs:
                bcs[nb] = sm_pool.tile([64, 512], F32, tag="bcast",
                                       name="bcast")
                nc.gpsimd.partition_broadcast(bcs[nb][:], rs[nb][:])
            for nb in nbs:
                nc.vector.tensor_mul(
                    o_sb[p][hh * 64:(hh + 1) * 64, nb * 512:(nb + 1) * 512],
                    o_ps[h][nb][0:64, :], bcs[nb][:])
            if nbs is None or list(nbs)[-1] == NB - 1:
                for key in [k for k in e_tiles if k[0] == h]:
                    del e_tiles[key]

        # --- step 2: head 0 S/exp + v projection + pair-1 q/k projections ---
        vps_cm = tc.tile_pool(name="vps", bufs=2, space="PSUM")
        vps = vps_cm.__enter__()
        p1ps_cm = tc.tile_pool(name="p1ps", bufs=2, space="PSUM")
        p1ps = p1ps_cm.__enter__()

        def emit_v(mt):
            ps = vps.tile([128, DG], F32, tag="vps", name="vpsn")
            for kc in range(KC):
                nc.tensor.matmul(
                    ps[:],
                    lhsT=xt_sb[kc][:, mt * 128:(mt + 1) * 128],
                    rhs=wv_sb[kc][:],
                    start=(kc == 0), stop=False)
            nc.tensor.matmul(
                ps[:], lhsT=ones[:, :128], rhs=bv_sb[:],
                start=False, stop=True)
            if USE_FP8_AV:
                dst = v_sb[mt // 2].rearrange(
                    "p (two h c) -> p two h c", two=2, c=VW)[:, mt % 2]
            else:
                dst = v_sb[mt].rearrange("p (h c) -> p h c", c=65)
            nc.vector.tensor_copy(dst[:, :, 0:64],
                                  ps.rearrange("p (h c) -> p h c", c=64))
            nc.vector.memset(dst[:, :, 64:65], 1.0)

        def emit_p1_group(i):
            """one (name, nb) accumulation group of the pair-1 projections."""
            name, nb = divmod(i, NB)
            w, bias, dst = ((wq_sb, bq_sb, qT_sb) if name == 0
                            else (wk_sb, bk_sb, kT_sb))
            ps = p1ps.tile([128, 512], F32, tag="p1", name="p1")
            for kc in range(KC):
                nc.tensor.matmul(
                    ps[:],
                    lhsT=w[kc][:, 128:256],
                    rhs=xt_sb[kc][:, nb * 512:(nb + 1) * 512],
                    start=(kc == 0), stop=(kc == KC - 1))
            nc.vector.tensor_scalar_add(
                dst[1][:, nb * 512:(nb + 1) * 512], ps[:], bias[:, 1:2])

        for mt in range(MT):
            emit_v(mt)
            emit_s_exp(0, mt)
            if mt % 2 == 1:
                emit_p1_group(mt // 2)

        p1ps_cm.__exit__(None, None, None)
        vps_cm.__exit__(None, None, None)

        o_pool_cm = tc.tile_pool(name="o_ps", bufs=4, space="PSUM")
        o_pool = o_pool_cm.__enter__()

        # --- steps 3-4: heads 1-2 S/exp + previous head's attn@v ------------
        for h in (1, 2):
            for mt in range(MT):
                emit_s_exp(h, mt)
                if USE_FP8_AV:
                    if mt % 2 == 1:
                        emit_av(h - 1, mt // 2, o_pool)
                else:
                    emit_av(h - 1, mt, o_pool)
            emit_norm(h - 1)

        # --- step 5: head 3 S/exp + attn@v of heads 2 and 3 -----------------
        for mt in range(MT):
            emit_s_exp(3, mt)
            if USE_FP8_AV:
                if mt < 8:
                    emit_av(2, mt, o_pool)
                    if mt == 7:
                        emit_norm(2)
                else:
                    emit_av(3, mt - 8, o_pool)
            else:
                if mt < 8:
                    emit_av(2, 2 * mt, o_pool)
                    emit_av(2, 2 * mt + 1, o_pool)
                    if mt == 7:
                        emit_norm(2)
                else:
                    emit_av(3, 2 * (mt - 8), o_pool)
                    emit_av(3, 2 * (mt - 8) + 1, o_pool)
        emit_norm(3)

        o_pool_cm.__exit__(None, None, None)
        s_pool_cm.__exit__(None, None, None)
        xpool_cm.__exit__(None, None, None)

        # --- output projection (partial: this group's rows of Wo.T) ---------
        # nb-outer so norm(3, nb) -> matmuls -> drains -> DMA pipeline per
        # column block; output in bf16 to halve the tail DMA.
        with (
            tc.tile_pool(name="out_ps", bufs=8, space="PSUM") as out_pool,
            tc.tile_pool(name="out_sb", bufs=8) as ostage,
        ):
            for nb in range(NB):
                for ft in range(FT):
                    ps = out_pool.tile([128, 512], F32, tag="outps",
                                       name="outps")
                    for pc in range(2):
                        nc.tensor.matmul(
                            ps[:],
                            lhsT=wo_sb[pc][:, ft * 128:(ft + 1) * 128],
                            rhs=o_sb[pc][:, nb * 512:(nb + 1) * 512],
                            start=(pc == 0), stop=(pc == 1))
                    stage = ostage.tile([128, 512], BF16, tag="ostage",
                                        name="ostage")
                    # both ScalarE and VectorE are idle by now; split drains
                    if ft % 2 == 0:
                        nc.scalar.copy(stage[:], ps[:])
                    else:
                        nc.vector.tensor_copy(stage[:], ps[:])
                    nc.sync.dma_start(
                        out=outT.ap()[ft * 128:(ft + 1) * 128,
                                      nb * 512:(nb + 1) * 512],
                        in_=stage[:])


_CACHED_NC = None


def _get_nc():
    global _CACHED_NC
    if _CACHED_NC is None:
        _CACHED_NC = build_kernel()
    return _CACHED_NC


def make_in_maps(x, Wq, bq, Wk, bk, Wv, bv, Wo, bo):
    """Host-side shard/layout prep: per-core input dict."""
    x = np.asarray(x, dtype=np.float32)
    xT_b = [np.ascontiguousarray(x[b].T).astype(NPBF16) for b in range(B)]
    WqT = np.asarray(Wq, np.float32).T.astype(NPBF16)  # [DIM(feat), DIM(out)]
    WkT = np.asarray(Wk, np.float32).T.astype(NPBF16)
    WvT = np.asarray(Wv, np.float32).T.astype(NPBF16)
    WoT = np.asarray(Wo, np.float32).T.astype(NPBF16)  # rows: concat feats
    bq = np.asarray(bq, np.float32)
    bk = np.asarray(bk, np.float32)
    bv16 = np.asarray(bv, np.float32).astype(NPBF16)

    in_maps = []
    for c in range(N_CORES):
        b, g = divmod(c, GROUPS)
        sl = slice(g * DG, (g + 1) * DG)
        in_maps.append({
            "xT": xT_b[b],
            "wqT": np.ascontiguousarray(WqT[:, sl]),
            "wkT": np.ascontiguousarray(WkT[:, sl]),
            "wvT": np.ascontiguousarray(WvT[:, sl]),
            "woT": np.ascontiguousarray(WoT[sl, :]),
            "bqc": np.ascontiguousarray(bq[sl].reshape(2, 128).T),
            "bkc": np.ascontiguousarray(bk[sl].reshape(2, 128).T),
            "bv": bv16[sl].reshape(1, DG),
        })
    return in_maps


def combine_outputs(results, bo):
    """Host-side unshard: sum group partials per batch, add bo."""
    bo = np.asarray(bo, np.float32)
    out = np.zeros((B, N, DIM), np.float32)
    for c in range(N_CORES):
        b = c // GROUPS
        out[b] += results[c]["outT"].astype(np.float32).T
    out += bo
    return out


def kernel(**inputs):
    nc = _get_nc()
    in_maps = make_in_maps(**{k: inputs[k] for k in
                              ("x", "Wq", "bq", "Wk", "bk", "Wv", "bv",
                               "Wo", "bo")})
    res = run_bass_kernel_spmd(nc, in_maps, list(range(N_CORES)))
    return combine_outputs(res.results, inputs["bo"])


if __name__ == "__main__":
    rng = np.random.default_rng(0)
    ins = {
        "x": rng.standard_normal((B, N, DIM), np.float32),
        "Wq": rng.standard_normal((DIM, DIM), np.float32) * 0.02,
        "bq": rng.standard_normal((DIM,), np.float32) * 0.02,
        "bk": rng.standard_normal((DIM,), np.float32) * 0.02,
        "Wk": rng.standard_normal((DIM, DIM), np.float32) * 0.02,
        "Wv": rng.standard_normal((DIM, DIM), np.float32) * 0.02,
        "bv": rng.standard_normal((DIM,), np.float32) * 0.02,
        "Wo": rng.standard_normal((DIM, DIM), np.float32) * 0.02,
        "bo": rng.standard_normal((DIM,), np.float32) * 0.02,
    }
    out = kernel(**ins)
    print("kernel output", out.shape, out.dtype, float(np.abs(out).mean()))
